# revision 18
# baseline (speedup 1.0000x reference)
"""DistMult edge scoring on TRN2 via transposed pair streaming + PE reduce.

Host does layout only (no arithmetic on values): casts h/W to bf16 (and an
fp8e4m3 copy for a small section of edges), sorts each core's edges by
relation, and materializes dense operand planes in a feature-on-partition
pair layout: column c holds edges 2c and 2c+1; partition p = 64*(edge parity)
+ feature. uplane carries h[src] rows, vplane h[dst]. Edges are split into a
bf16 section and a small fp8 section (error headroom vs the 2e-2 gate);
relation runs are padded to whole 128-slot boundaries, shared across cores.

Device per core: stream plane tiles (dense DMA, no gather descriptors; two
dma_starts per tile keep the HWDGE ring full). DVE does ONE fused pass
q = (u * w_ptr) * v via scalar_tensor_tensor (w_ptr = per-partition scalar
W[r, p%64] per relation run); fp8 tiles write q as bf16 into the spare bf16
tile buffer. PE reduces the 64 features per edge with matmuls against a fixed
[128,2] halves-summing stationary into PSUM [2, 512] bank chunks, grouped in
2048-column halves (ping-pong). ACT evacuates each group with fused Sigmoid
into bf16 scores (host casts back to f32); gpsimd issues the
output stores. Tail tiles shrink to cut drain
latency. Host unpermutes.
"""

import sys

sys.path.insert(0, "/opt/trn_rl_repo")

import numpy as np
import ml_dtypes

N_NODES = 500000
N_HID = 64
N_RELS = 10
N_CORES = 8
P = 128
TCC = 4096  # max columns (edge pairs) per DMA tile
MM = 512  # columns per matmul chunk (PSUM bank)
GV = 2048  # columns per PSUM group / ACT evacuation (4 banks)
FRAC8 = 0.21  # fraction of edges routed to the fp8 section


def _tile_list(SL, tail):
    """Split SL (multiple of GV//4) into tiles of <=TCC columns; with
    tail=True the final columns shrink to 2048/1024/512/512 to cut drain
    latency."""
    assert SL % (GV // 4) == 0
    tiles = []
    rem = SL
    while rem >= TCC + (TCC if tail else 0):
        tiles.append(TCC)
        rem -= TCC
    if tail:
        for c in (TCC // 2, TCC // 4, TCC // 8, TCC // 8):
            if rem >= c and c >= GV // 4:
                tiles.append(c)
                rem -= c
        while rem:
            tiles.append(min(rem, GV // 4))
            rem -= tiles[-1]
    else:
        while rem:
            tiles.append(min(rem, TCC))
            rem -= tiles[-1]
    assert sum(tiles) == SL
    return tiles


def _build_program(LA, L8, n_bufs=10, n_ev=4):
    """LA/L8: per-relation padded slot counts for the bf16 / fp8 sections
    (multiples of 128; each section's total a multiple of 2*GV)."""
    from contextlib import ExitStack

    from concourse import bass, bacc, mybir

    f32 = mybir.dt.float32
    bf16 = mybir.dt.bfloat16
    f8 = mybir.dt.float8e4

    LA = [int(x) for x in LA]
    L8 = [int(x) for x in L8]
    SLA = sum(LA) // 2
    SL8 = sum(L8) // 2
    SL = SLA + SL8
    B = n_bufs

    # global tile list: (section, col offset in section, cols); fp8 first so
    # the stream ends on the bf16 section (higher DMA-per-compute ratio)
    tl_8 = _tile_list(SL8, tail=False)
    tl_a = _tile_list(SLA, tail=True)
    tdesc = []
    off = 0
    for c in tl_8:
        tdesc.append((1, off, c))
        off += c
    off = 0
    for c in tl_a:
        tdesc.append((0, off, c))
        off += c
    T = len(tdesc)
    gbase = []  # global col base per tile
    for sec, off, c in tdesc:
        gbase.append(off + (0 if sec else SL8))

    # relation col bounds per section (section-local)
    rc_a = np.concatenate([[0], np.cumsum(LA) // 2]).astype(int)
    rc_8 = np.concatenate([[0], np.cumsum(L8) // 2]).astype(int)

    # per-tile relation segments (c0, c1, r), columns relative to tile
    tsegs = []
    for sec, off, c in tdesc:
        rc = rc_8 if sec else rc_a
        segs = []
        for r in range(N_RELS):
            a, b = max(off, rc[r]), min(off + c, rc[r + 1])
            if a < b:
                segs.append((a - off, b - off, r))
        tsegs.append(segs)

    # evacuation groups: (tile, col offset in tile, cols); every group <= GV
    groups = []
    for t in range(T):
        off = 0
        while off < tdesc[t][2]:
            g = min(GV, tdesc[t][2] - off)
            groups.append((t, off, g))
            off += g
    NG = len(groups)
    gchunk = [0]
    for t, off, g in groups:
        gchunk.append(gchunk[-1] + g // MM)
    tile_last_group = {}
    for gi, (t, off, g) in enumerate(groups):
        tile_last_group[t] = gi

    nc = bacc.Bacc("TRN2")
    upsA = nc.declare_dram_parameter("upsA", [P, SLA], bf16, isOutput=False)
    vpsA = nc.declare_dram_parameter("vpsA", [P, SLA], bf16, isOutput=False)
    ups8 = nc.declare_dram_parameter("ups8", [P, SL8], f8, isOutput=False)
    vps8 = nc.declare_dram_parameter("vps8", [P, SL8], f8, isOutput=False)
    wcol = nc.declare_dram_parameter("wcol", [P, N_RELS], f32, isOutput=False)
    lhs = nc.declare_dram_parameter("lhs", [P, 2], bf16, isOutput=False)
    out = nc.declare_dram_parameter("out", [2, SL], bf16, isOutput=True)

    with ExitStack() as es:
        pre = es.enter_context(nc.semaphore("pre"))
        dma_sems = [es.enter_context(nc.semaphore(f"dma{i}")) for i in range(B)]
        dve_sem = es.enter_context(nc.semaphore("dve_sem"))
        pe_sem = es.enter_context(nc.semaphore("pe_sem"))
        act_sem = es.enter_context(nc.semaphore("act_sem"))
        st_sem = es.enter_context(nc.semaphore("st_sem"))
        w_sb = es.enter_context(nc.sbuf_tensor("w_sb", [P, N_RELS], f32))
        lhs_sb = es.enter_context(nc.sbuf_tensor("lhs_sb", [P, 2], bf16))
        ev_sb = [
            es.enter_context(nc.sbuf_tensor(f"ev{i}", [2, GV], bf16))
            for i in range(n_ev)
        ]
        u_sb = [
            es.enter_context(nc.sbuf_tensor(f"u{i}", [P, TCC], bf16)) for i in range(B)
        ]
        v_sb = [
            es.enter_context(nc.sbuf_tensor(f"v{i}", [P, TCC], bf16)) for i in range(B)
        ]
        psum = es.enter_context(nc.psum_tensor("psq", [P, 2 * GV], f32))

        with nc.Block() as block:

            @block.sync
            def _(sync):
                for t, (sec, off, cols) in enumerate(tdesc):
                    if t >= B:
                        lg = tile_last_group[t - B]
                        sync.wait_ge(pe_sem, gchunk[lg + 1])
                    if sec == 0:
                        du = u_sb[t % B][:, :cols]
                        dv = v_sb[t % B][:, :cols]
                        pu, pv = upsA, vpsA
                    else:
                        du = v_sb[t % B][:, : cols // 2].bitcast(f8)
                        dv = v_sb[t % B][
                            :, TCC // 2 : TCC // 2 + cols // 2
                        ].bitcast(f8)
                        pu, pv = ups8, vps8
                    sync.dma_start(
                        out=du, in_=pu[:, off : off + cols]
                    ).then_inc(dma_sems[t % B], 16)
                    sync.dma_start(
                        out=dv, in_=pv[:, off : off + cols]
                    ).then_inc(dma_sems[t % B], 16)

            @block.vector
            def _(dve):
                dve.wait_ge(pre, 32)
                mult = mybir.AluOpType.mult
                for t, (sec, off, cols) in enumerate(tdesc):
                    dve.wait_ge(dma_sems[t % B], 32 * (t // B + 1))
                    if sec == 1 and t >= B:
                        # q overwrites u_sb[t%B], which holds tile t-B's
                        # products until PE consumed them
                        lg = tile_last_group[t - B]
                        dve.wait_ge(pe_sem, gchunk[lg + 1])
                    last = None
                    for c0, c1, r in tsegs[t]:
                        if sec == 0:
                            in0 = u_sb[t % B][:, c0:c1]
                            in1 = v_sb[t % B][:, c0:c1]
                        else:
                            in0 = v_sb[t % B][:, c0 // 2 : c1 // 2].bitcast(f8)
                            in1 = v_sb[t % B][
                                :, TCC // 2 + c0 // 2 : TCC // 2 + c1 // 2
                            ].bitcast(f8)
                        last = dve.scalar_tensor_tensor(
                            out=u_sb[t % B][:, c0:c1],
                            in0=in0,
                            scalar=w_sb[:, r : r + 1],
                            in1=in1,
                            op0=mult,
                            op1=mult,
                        )
                    last.then_inc(dve_sem, 1)

            @block.tensor
            def _(pe):
                for gi, (t, off, g) in enumerate(groups):
                    pe.wait_ge(dve_sem, t + 1)
                    if gi >= 2:
                        pe.wait_ge(act_sem, gi - 1)
                    p0 = (gi % 2) * GV
                    for k in range(g // MM):
                        pe.matmul(
                            psum[0:2, p0 + k * MM : p0 + (k + 1) * MM],
                            lhs_sb[:],
                            u_sb[t % B][:, off + k * MM : off + (k + 1) * MM],
                        ).then_inc(pe_sem, 1)

            @block.scalar
            def _(act):
                for gi, (t, off, g) in enumerate(groups):
                    act.wait_ge(pe_sem, gchunk[gi + 1])
                    if gi >= n_ev:
                        act.wait_ge(st_sem, 16 * (gi - n_ev + 1))
                    p0 = (gi % 2) * GV
                    act.activation(
                        out=ev_sb[gi % n_ev][:, :g],
                        in_=psum[0:2, p0 : p0 + g],
                        func=mybir.ActivationFunctionType.Sigmoid,
                    ).then_inc(act_sem, 1)

            @block.gpsimd
            def _(gp):
                gp.dma_start(out=w_sb[:], in_=wcol[:]).then_inc(pre, 16)
                gp.dma_start(out=lhs_sb[:], in_=lhs[:]).then_inc(pre, 16)
                for gi, (t, off, g) in enumerate(groups):
                    c0 = gbase[t] + off
                    gp.wait_ge(act_sem, gi + 1)
                    gp.dma_start(
                        out=out[:, c0 : c0 + g], in_=ev_sb[gi % n_ev][:, :g]
                    ).then_inc(st_sem, 16)
                gp.wait_ge(st_sem, 16 * NG)

    nc.compile()
    return nc


def _section_layout(rel_sec, counts_list):
    """Shared padded per-rel slot counts for one section; returns L."""
    Lmax = np.maximum.reduce(counts_list)
    L = ((Lmax + P - 1) // P) * P
    tot = int(L.sum())
    grain = GV // 2  # 1024 slots = 512 cols, the matmul-chunk granularity
    padded = ((tot + grain - 1) // grain) * grain
    padded = max(padded, 4 * GV)
    L[-1] += padded - tot
    return L


def _plane(rows, SL):
    """[Etot, 64] rows -> [128, SL] feature-on-partition pair plane."""
    return np.ascontiguousarray(
        rows.reshape(SL, 2, N_HID).transpose(1, 2, 0).reshape(P, SL)
    )


def kernel(h, W, src_idx, dst_idx, rel_idx):
    from concourse.bass_utils import run_bass_kernel_spmd

    bf16 = ml_dtypes.bfloat16
    f8 = ml_dtypes.float8_e4m3
    h_f = np.asarray(h, dtype=np.float32)
    h_bf = h_f.astype(bf16)
    h_f8 = h_f.astype(f8)
    W_f = np.asarray(W, dtype=np.float32)
    src = np.asarray(src_idx).astype(np.int64)
    dst = np.asarray(dst_idx).astype(np.int64)
    rel = np.asarray(rel_idx).astype(np.int64)

    E = src.shape[0]
    esh = E // N_CORES
    k8 = int(esh * FRAC8)
    ka = esh - k8  # first ka edges -> bf16 section, rest -> fp8 section

    per_core = []
    counts_a, counts_8 = [], []
    for i in range(N_CORES):
        sl = slice(i * esh, (i + 1) * esh)
        r_c = rel[sl]
        oa = np.argsort(r_c[:ka], kind="stable")
        o8 = ka + np.argsort(r_c[ka:], kind="stable")
        counts_a.append(np.bincount(r_c[:ka], minlength=N_RELS))
        counts_8.append(np.bincount(r_c[ka:], minlength=N_RELS))
        per_core.append((oa, o8))

    LA = _section_layout(0, counts_a)
    L8 = _section_layout(1, counts_8)
    EtotA, Etot8 = int(LA.sum()), int(L8.sum())
    SLA, SL8 = EtotA // 2, Etot8 // 2
    SL = SLA + SL8
    base_a = np.concatenate([[0], np.cumsum(LA)]).astype(int)
    base_8 = np.concatenate([[0], np.cumsum(L8)]).astype(int)

    wcol = np.ascontiguousarray(np.tile(W_f.T, (2, 1)))  # [128, 10] f32
    lhs = np.zeros((P, 2), dtype=bf16)
    lhs[:N_HID, 0] = 1
    lhs[N_HID:, 1] = 1

    in_maps, metas = [], []
    for i in range(N_CORES):
        sl = slice(i * esh, (i + 1) * esh)
        s_c, d_c, r_c = src[sl], dst[sl], rel[sl]
        oa, o8 = per_core[i]

        def build(order, counts, rel_base, Etot, SLs, htab):
            srt_s = s_c[order]
            srt_d = d_c[order]
            cnt = counts
            starts = np.concatenate([[0], np.cumsum(cnt[:-1])])
            ranks = np.arange(order.shape[0]) - np.repeat(starts, cnt)
            slots = np.repeat(rel_base[:-1], cnt) + ranks
            ru = np.zeros((Etot, N_HID), dtype=htab.dtype)
            rv = np.zeros((Etot, N_HID), dtype=htab.dtype)
            ru[slots] = htab[srt_s]
            rv[slots] = htab[srt_d]
            return _plane(ru, SLs), _plane(rv, SLs), slots

        upsA, vpsA, slots_a = build(oa, counts_a[i], base_a, EtotA, SLA, h_bf)
        ups8, vps8, slots_8 = build(o8, counts_8[i], base_8, Etot8, SL8, h_f8)
        in_maps.append(
            {
                "upsA": upsA,
                "vpsA": vpsA,
                "ups8": ups8,
                "vps8": vps8,
                "wcol": wcol,
                "lhs": lhs,
            }
        )
        order_all = np.concatenate([oa, o8])
        slots_all = np.concatenate([2 * SL8 + slots_a, slots_8])
        metas.append((order_all, slots_all))

    key = (tuple(int(x) for x in LA), tuple(int(x) for x in L8))
    if key not in _PROGRAM_CACHE:
        _PROGRAM_CACHE[key] = _build_program(LA, L8)
    nc = _PROGRAM_CACHE[key]

    res = run_bass_kernel_spmd(
        nc, in_maps, core_ids=list(range(N_CORES)), trace=TRACE
    )
    global LAST_RESULT
    LAST_RESULT = res

    out_full = np.empty(E, dtype=np.float32)
    for i in range(N_CORES):
        arr = np.asarray(res.results[i]["out"]).astype(np.float32)  # [2, SL]
        s_lin = arr.T.reshape(-1)  # slot j = (j%2, j//2) -> arr[par, c]
        order, slots = metas[i]
        out_full[i * esh + order] = s_lin[slots]
    return out_full


_PROGRAM_CACHE = {}
TRACE = False
LAST_RESULT = None
